# revision 1
# baseline (speedup 1.0000x reference)
"""Trainium2 Bass kernel for nn_BertAoA_Decoder_Core (6-layer BERT-style encoder,
layer-0 cross-attention to p_att_feats).

Strategy: pure data-parallel over batch across 8 NeuronCores (8 examples/core).
All activations stay SBUF-resident for the whole 6-layer stack; weights stream
from HBM under PE compute.  Host-side algebraic folding removes LN affine params
and most biases; attention runs in bf16, FFN/LN/residual in fp32 with float32r
matmuls (full PE rate at N=512).
"""

import sys

sys.path.insert(0, "/opt/trn_rl_repo")

import numpy as np
import ml_dtypes
from contextlib import ExitStack

import concourse.bass as bass
import concourse.mybir as mybir
import concourse.tile as tile
from concourse.masks import make_identity
from concourse.bass_utils import run_bass_kernel_spmd

F32 = mybir.dt.float32
F32R = mybir.dt.float32r
BF16 = mybir.dt.bfloat16
I32 = mybir.dt.int32
AX = mybir.AxisListType.X
OP = mybir.AluOpType
ACT = mybir.ActivationFunctionType

# Problem constants (hardcoded per contract)
B, S, C, D, H, L, F = 64, 128, 196, 1024, 16, 6, 4096
DK = D // H              # 64
NCORES = 8
BL = B // NCORES         # 8 examples per core
T = BL * S               # 1024 query tokens per core
TC0 = BL * C             # 1568 context tokens per core (layer 0)
KD = D // 128            # 8 contraction tiles
MD = D // 128            # 8 output tiles
FT = F // 128            # 32 FFN tiles
NFB = 4                  # FFN f-blocks
FBT = FT // NFB          # 8 f-tiles per block
NGRP = 2                 # example groups per core
GE = BL // NGRP          # 4 examples per group
GT = GE * S              # 512 tokens per group
GC = GE * C              # 784 context tokens per group (layer 0)
LN_EPS = 1e-6


def _split_multi_waits(nc):
    """This container's walrus accepts only one sync-wait per CTRL instruction;
    hoist extra waits onto preceding NoOps on the same engine."""
    cnt = 0
    for fn in nc.m.functions:
        for bb in fn.blocks:
            new_list = []
            for ins in bb.instructions:
                si = getattr(ins, "sync_info", None)
                ow = getattr(si, "on_wait", None) if si is not None else None
                if ow and len(ow) > 1:
                    for w in ow[:-1]:
                        nop = mybir.InstNoOp(
                            name=f"{ins.name}-wsplit-{cnt}",
                            engine=ins.engine,
                            sync_info=mybir.SyncInfo(on_wait=[w], on_update=[]),
                        )
                        cnt += 1
                        new_list.append(nop)
                    si.on_wait = [ow[-1]]
                new_list.append(ins)
            bb.instructions = new_list
    return cnt


def _newton_rsqrt(nc, pool, v_ap, out_ap, n):
    """out = 1/sqrt(v) elementwise on a small [128, n] fp32 AP, DVE-only.

    y0 = 0.5*(1 + 1/v) (good near v~1, converges for v in ~[0.15, 5.5] which
    covers LayerNorm variances here), then 4 Newton iterations
    y <- y*(1.5 - 0.5*v*y^2)."""
    r = pool.tile([128, n], F32, name="rs_r", tag="rs_r")
    t = pool.tile([128, n], F32, name="rs_t", tag="rs_t")
    nc.vector.reciprocal(r, v_ap)
    nc.vector.tensor_scalar(out_ap, r, 0.5, 0.5, OP.mult, OP.add)
    for _ in range(4):
        nc.vector.tensor_tensor(t, out_ap, out_ap, OP.mult)      # y^2
        nc.vector.tensor_tensor(t, t, v_ap, OP.mult)             # v*y^2
        nc.vector.tensor_scalar(t, t, -0.5, 1.5, OP.mult, OP.add)
        nc.vector.tensor_tensor(out_ap, out_ap, t, OP.mult)


def _layer_norm(nc, stats_pool, x_tiles, h_pool, out_dtype, tagpfx):
    """Pre-norm (x-mu)*rstd for 8 [128, D] token-major tiles (affine folded
    into the weights host-side).  Processed in two half-batches so the second
    half's stats can start before the first half's consumers finish."""
    h_tiles = [None] * BL
    for hb in range(2):
        i0 = hb * (BL // 2)
        nb = BL // 2
        stat = stats_pool.tile([128, nb, 12], F32, name=f"{tagpfx}_stat{hb}",
                               tag=f"{tagpfx}_stat")
        mv = stats_pool.tile([128, nb, 2], F32, name=f"{tagpfx}_mv{hb}",
                             tag=f"{tagpfx}_mv")
        var = stats_pool.tile([128, nb], F32, name=f"{tagpfx}_var{hb}",
                              tag=f"{tagpfx}_var")
        rst = stats_pool.tile([128, nb], F32, name=f"{tagpfx}_rst{hb}",
                              tag=f"{tagpfx}_rst")
        for i in range(nb):
            nc.vector.bn_stats(stat[:, i, 0:6], x_tiles[i0 + i][:, 0:512])
            nc.vector.bn_stats(stat[:, i, 6:12], x_tiles[i0 + i][:, 512:1024])
            nc.vector.bn_aggr(mv[:, i, :], stat[:, i, :])
        nc.vector.tensor_scalar(var, mv[:, :, 1], LN_EPS, None, OP.add)
        _newton_rsqrt(nc, stats_pool, var, rst, nb)
        for i in range(nb):
            h = h_pool.tile([128, D], out_dtype, name=f"{tagpfx}_h{i0+i}",
                            tag=f"{tagpfx}_h")
            nc.vector.tensor_scalar(h, x_tiles[i0 + i], mv[:, i, 0:1],
                                    rst[:, i : i + 1], OP.subtract, OP.mult)
            h_tiles[i0 + i] = h
    return h_tiles


def _transpose_to_fm(nc, tpsum, h_tm, fm_pool, dtype, ident, tagpfx, ncols=T,
                     fm_dtype=None):
    """Token-major tiles [128, D] -> feature-major tiles fm[k][128, ncols].
    Four [128,128] transposes pack one [128,512] psum, evicted in one op."""
    fm = [fm_pool.tile([128, ncols], fm_dtype or dtype, name=f"{tagpfx}_fm{k}",
                       tag=f"{tagpfx}_fm")
          for k in range(KD)]
    for i0 in range(0, BL, 4):
        for k in range(KD):
            ps = tpsum.tile([128, 512], dtype, name=f"{tagpfx}_tp4", tag="tp4")
            for i in range(i0, i0 + 4):
                nc.tensor.transpose(ps[:, (i - i0) * 128 : (i - i0 + 1) * 128],
                                    h_tm[i][:, k * 128 : (k + 1) * 128], ident)
            nc.vector.tensor_copy(fm[k][:, i0 * 128 : (i0 + 4) * 128], ps)
    return fm


def _mm_accum(nc, ps, pairs, f32r=False):
    n = len(pairs)
    for i, (lhsT, rhs) in enumerate(pairs):
        nc.tensor.matmul(ps, lhsT, rhs, start=(i == 0), stop=(i == n - 1))


def build_program(nonzero_bo, nonzero_b2, n_layers=L):
    nc = bass.Bass()
    x_in = nc.declare_dram_parameter("x", [T, D], F32, isOutput=False)
    y_out = nc.declare_dram_parameter("y", [T, D], F32, isOutput=True)
    kv0 = nc.declare_dram_parameter("kv0", [KD, 128, TC0], BF16, isOutput=False)
    wq_d = nc.declare_dram_parameter("wq", [L, MD, 128, KD * 128], BF16, isOutput=False)
    wk_d = nc.declare_dram_parameter("wk", [L, MD, 128, KD * 128], BF16, isOutput=False)
    wv_d = nc.declare_dram_parameter("wv", [L, KD, 128, D], BF16, isOutput=False)
    wo_d = nc.declare_dram_parameter("wo", [L, KD, 128, D], BF16, isOutput=False)
    w1_d = nc.declare_dram_parameter("w1", [L, FT, 128, KD * 128], F32R, isOutput=False)
    w2_d = nc.declare_dram_parameter("w2", [L, FT, 128, D], F32R, isOutput=False)
    bq_d = nc.declare_dram_parameter("bq", [L, 128, MD], F32, isOutput=False)
    b1_d = nc.declare_dram_parameter("b1", [L, 128, FT], F32, isOutput=False)
    if nonzero_bo:
        bo_d = nc.declare_dram_parameter("bo_bc", [L, 128, D], F32, isOutput=False)
    if nonzero_b2:
        b2_d = nc.declare_dram_parameter("b2_bc", [L, 128, D], F32, isOutput=False)

    with tile.TileContext(nc) as tc, ExitStack() as top:
        const = top.enter_context(tc.tile_pool(name="const", bufs=1))
        ident_bf = const.tile([128, 128], BF16, name="ident_bf")
        make_identity(nc, ident_bf)
        ident_f32 = const.tile([128, 128], F32, name="ident_f32")
        make_identity(nc, ident_f32)

        xpool = top.enter_context(tc.tile_pool(name="xres", bufs=BL))
        xt = []
        for i in range(BL):
            t_ = xpool.tile([128, D], F32, name=f"x{i}", tag="x")
            nc.sync.dma_start(t_, x_in[i * 128 : (i + 1) * 128, :])
            xt.append(t_)

        for l in range(n_layers):
            # ---------------- attention sublayer ----------------
            with ExitStack() as actx:
                stats = actx.enter_context(tc.tile_pool(name=f"l{l}_st", bufs=1))
                h1tm_p = actx.enter_context(tc.tile_pool(name=f"l{l}_h1tm", bufs=4))
                h1fm_p = actx.enter_context(tc.tile_pool(name=f"l{l}_h1fm", bufs=KD))
                wqk_p = actx.enter_context(tc.tile_pool(name=f"l{l}_wqk", bufs=16))
                wvo_p = actx.enter_context(tc.tile_pool(name=f"l{l}_wvo", bufs=16))
                gbufs = 1 if l == 0 else 2
                qa_p = actx.enter_context(tc.tile_pool(name=f"l{l}_qa", bufs=gbufs))
                kg_p = actx.enter_context(tc.tile_pool(name=f"l{l}_kg", bufs=gbufs))
                v_p = actx.enter_context(tc.tile_pool(name=f"l{l}_v", bufs=8))
                ag_p = actx.enter_context(tc.tile_pool(name=f"l{l}_ag", bufs=gbufs))
                sm_p = actx.enter_context(tc.tile_pool(name=f"l{l}_sm", bufs=2))
                sc_p = actx.enter_context(tc.tile_pool(name=f"l{l}_sc", bufs=4))
                bias_p = actx.enter_context(tc.tile_pool(name=f"l{l}_bias", bufs=1))
                ppsum = actx.enter_context(
                    tc.tile_pool(name=f"l{l}_pps", bufs=2, space="PSUM"))
                spsum = actx.enter_context(
                    tc.tile_pool(name=f"l{l}_sps", bufs=2, space="PSUM"))
                tpsum = actx.enter_context(
                    tc.tile_pool(name=f"l{l}_tps", bufs=2, space="PSUM"))
                apsum = actx.enter_context(
                    tc.tile_pool(name=f"l{l}_aps", bufs=2, space="PSUM"))
                if l == 0:
                    kv0_p = actx.enter_context(tc.tile_pool(name="l0_kv0", bufs=KD))

                bqt = bias_p.tile([128, MD], F32, name=f"l{l}_bqt")
                nc.sync.dma_start(bqt, bq_d[l])

                # LayerNorm 1 (token-major, bf16 out) and transpose to fm
                h1tm = _layer_norm(nc, stats, xt, h1tm_p, BF16, f"l{l}a")
                h1fm = _transpose_to_fm(nc, tpsum, h1tm, h1fm_p, BF16,
                                        ident_bf, f"l{l}a")

                # weight tiles for the whole layer
                wq_t = []
                wk_t = []
                for m in range(MD):
                    w = wqk_p.tile([128, KD * 128], BF16, name=f"l{l}_wq{m}", tag="wqk")
                    nc.sync.dma_start(w, wq_d[l, m])
                    wq_t.append(w)
                for m in range(MD):
                    w = wqk_p.tile([128, KD * 128], BF16, name=f"l{l}_wk{m}", tag="wqk")
                    nc.sync.dma_start(w, wk_d[l, m])
                    wk_t.append(w)
                wv_t = []
                wo_t = []
                for k in range(KD):
                    w = wvo_p.tile([128, D], BF16, name=f"l{l}_wv{k}", tag="wvo")
                    nc.sync.dma_start(w, wv_d[l, k])
                    wv_t.append(w)
                for k in range(KD):
                    w = wvo_p.tile([128, D], BF16, name=f"l{l}_wo{k}", tag="wvo")
                    nc.sync.dma_start(w, wo_d[l, k])
                    wo_t.append(w)

                TCB = C if l == 0 else S          # context length per example
                TCG = GE * TCB                    # per group

                for g in range(NGRP):
                    gcol = slice(g * GT, (g + 1) * GT)
                    if l == 0:
                        kvg = []
                        for k in range(KD):
                            kt = kv0_p.tile([128, GC], BF16, name=f"kv0_{k}", tag="kv0")
                            nc.sync.dma_start(kt, kv0[k, :, g * GC : (g + 1) * GC])
                            kvg.append(kt)
                    else:
                        kvg = None

                    # ---- Q projection (feature-major, pre-scaled, +bq) ----
                    qg = qa_p.tile([128, MD, GT], BF16, name=f"l{l}g{g}_q", tag="qg")
                    for m in range(MD):
                        ps = ppsum.tile([128, 512], F32, name="qps", tag="pps")
                        _mm_accum(nc, ps,
                                  [(wq_t[m][:, k * 128 : (k + 1) * 128],
                                    h1fm[k][:, gcol]) for k in range(KD)])
                        nc.scalar.activation(qg[:, m, :], ps, ACT.Identity,
                                             bias=bqt[:, m : m + 1])
                    # ---- K projection ----
                    kg = kg_p.tile([128, MD, TCG], BF16, name=f"l{l}g{g}_k", tag="kg")
                    ksrc = kvg if l == 0 else [h1fm[k][:, gcol] for k in range(KD)]
                    for m in range(MD):
                        for n0 in range(0, TCG, 512):
                            n1 = min(n0 + 512, TCG)
                            ps = ppsum.tile([128, 512], F32, name="kps", tag="pps")
                            _mm_accum(nc, ps[:, : n1 - n0],
                                      [(wk_t[m][:, k * 128 : (k + 1) * 128],
                                        ksrc[k][:, n0:n1]) for k in range(KD)])
                            nc.vector.tensor_copy(kg[:, m, n0:n1], ps[:, : n1 - n0])
                    # ---- V projection (token-major, per example) ----
                    vts = []      # per example: list of (tile, nrows)
                    for e in range(GE):
                        segs = []
                        for s0 in range(0, TCB, 128):
                            nrows = min(128, TCB - s0)
                            vt = v_p.tile([128, D], BF16, name=f"l{l}g{g}e{e}v{s0}",
                                          tag="v")
                            for n in range(2):
                                ps = ppsum.tile([128, 512], F32, name="vps", tag="pps")
                                if l == 0:
                                    lh = [(kvg[k][:, e * TCB + s0 : e * TCB + s0 + nrows],
                                           wv_t[k][:, n * 512 : (n + 1) * 512])
                                          for k in range(KD)]
                                else:
                                    c0 = (g * GE + e) * 128
                                    lh = [(h1fm[k][:, c0 : c0 + 128],
                                           wv_t[k][:, n * 512 : (n + 1) * 512])
                                          for k in range(KD)]
                                _mm_accum(nc, ps[: nrows], lh)
                                nc.vector.tensor_copy(
                                    vt[:nrows, n * 512 : (n + 1) * 512], ps[:nrows])
                            segs.append((vt, nrows))
                        vts.append(segs)

                    # ---- attention smalls, batched by head groups ----
                    # Scores use per-head offset-0 psums (K=64 stationaries
                    # corrupt column-offset psum writes on this silicon).  The
                    # softmax batches bh heads via SBUF slice writes: shared
                    # max across the batch, segmented sum, one reciprocal, one
                    # 0-stride-broadcast normalize.  p-transposes (K=128) pack
                    # one [128,512] psum -> per-segment eviction.
                    ag = ag_p.tile([128, MD, GT], BF16, name=f"l{l}g{g}_a", tag="ag")
                    bh = 512 // TCB          # 4 heads (S=128) or 2 heads (C=196)
                    nseg = (TCB + 127) // 128
                    for e in range(GE):
                        for hb in range(0, H, bh):
                            # exp without max-subtraction: LN'd activations and
                            # 0.02-scale weights bound |scores| << 80, so fp32
                            # exp cannot overflow; softmax is shift-free here.
                            praw = sm_p.tile([128, bh, TCB], F32, name="praw", tag="praw")
                            for hi in range(bh):
                                h_ = hb + hi
                                po, ch = 64 * (h_ % 2), h_ // 2
                                sp = spsum.tile([128, TCB], F32, name="sp", tag="sps")
                                nc.tensor.matmul(
                                    sp,
                                    qg[po : po + 64, ch, e * 128 : (e + 1) * 128],
                                    kg[po : po + 64, ch, e * TCB : (e + 1) * TCB],
                                    start=True, stop=True)
                                nc.scalar.activation(praw[:, hi, :], sp, ACT.Exp)
                            ssum4 = sc_p.tile([128, bh], F32, name="ssum4", tag="ssum")
                            nc.vector.tensor_reduce(ssum4, praw, AX, OP.add)
                            rinv4 = sc_p.tile([128, bh], F32, name="rinv4", tag="rinv")
                            nc.vector.reciprocal(rinv4, ssum4)
                            pbf = sm_p.tile([128, bh, TCB], BF16, name="pbf", tag="pbf")
                            nc.vector.tensor_tensor(
                                pbf, praw,
                                rinv4[:, :, None].broadcast_to((128, bh, TCB)),
                                OP.mult)
                            # transpose all bh*nseg p-blocks into one psum bank
                            # (segment-major so evictions touch only written rows)
                            tp4 = tpsum.tile([128, nseg, bh, 128], BF16,
                                             name="ptp4", tag="tp4")
                            for hi in range(bh):
                                for si in range(nseg):
                                    nrows = min(128, TCB - si * 128)
                                    nc.tensor.transpose(
                                        tp4[:nrows, si, hi, :],
                                        pbf[:, hi, si * 128 : si * 128 + nrows],
                                        ident_bf)
                            pts = sm_p.tile([128, nseg, bh, 128], BF16,
                                            name="pts", tag="pts")
                            for si in range(nseg):
                                nrows = min(128, TCB - si * 128)
                                nc.scalar.activation(pts[:nrows, si], tp4[:nrows, si],
                                                     ACT.Copy)
                            for hi in range(bh):
                                h_ = hb + hi
                                po, ch = 64 * (h_ % 2), h_ // 2
                                aps = apsum.tile([64, 128], F32, name="aps", tag="aps")
                                for si in range(nseg):
                                    nrows = min(128, TCB - si * 128)
                                    vt, _ = vts[e][si]
                                    nc.tensor.matmul(
                                        aps, vt[:nrows, h_ * 64 : (h_ + 1) * 64],
                                        pts[:nrows, si, hi, :],
                                        start=(si == 0), stop=(si == nseg - 1))
                                nc.scalar.activation(
                                    ag[po : po + 64, ch, e * 128 : (e + 1) * 128],
                                    aps, ACT.Copy)
                    # ---- output projection, residual add ----
                    for e in range(GE):
                        xi = xt[g * GE + e]
                        for n in range(2):
                            ps = ppsum.tile([128, 512], F32, name="ops", tag="pps")
                            _mm_accum(nc, ps,
                                      [(ag[:, k, e * 128 : (e + 1) * 128],
                                        wo_t[k][:, n * 512 : (n + 1) * 512])
                                       for k in range(KD)])
                            nc.vector.tensor_tensor(
                                xi[:, n * 512 : (n + 1) * 512],
                                xi[:, n * 512 : (n + 1) * 512], ps, OP.add)
                if nonzero_bo:
                    bo_t = bias_p.tile([128, D], F32, name=f"l{l}_bo")
                    nc.sync.dma_start(bo_t, bo_d[l])
                    for i in range(BL):
                        nc.vector.tensor_tensor(xt[i], xt[i], bo_t, OP.add)

            # ---------------- FFN sublayer ----------------
            with ExitStack() as fctx:
                stats2 = fctx.enter_context(tc.tile_pool(name=f"l{l}_st2", bufs=1))
                h2tm_p = fctx.enter_context(tc.tile_pool(name=f"l{l}_h2tm", bufs=4))
                h2fm_p = fctx.enter_context(tc.tile_pool(name=f"l{l}_h2fm", bufs=KD))
                w1_p = fctx.enter_context(tc.tile_pool(name=f"l{l}_w1", bufs=10))
                w2_p = fctx.enter_context(tc.tile_pool(name=f"l{l}_w2", bufs=10))
                u_p = fctx.enter_context(tc.tile_pool(name=f"l{l}_u", bufs=12))
                bias2_p = fctx.enter_context(tc.tile_pool(name=f"l{l}_b2", bufs=1))
                fpsum = fctx.enter_context(
                    tc.tile_pool(name=f"l{l}_fps", bufs=4, space="PSUM"))
                tpsum2 = fctx.enter_context(
                    tc.tile_pool(name=f"l{l}_tps2", bufs=2, space="PSUM"))

                b1t = bias2_p.tile([128, FT], F32, name=f"l{l}_b1t")
                nc.sync.dma_start(b1t, b1_d[l])

                h2tm = _layer_norm(nc, stats2, xt, h2tm_p, F32, f"l{l}f")
                h2fm = _transpose_to_fm(nc, tpsum2, h2tm, h2fm_p, F32,
                                        ident_f32, f"l{l}f", fm_dtype=F32R)

                for fb in range(NFB):
                    w1t = []
                    w2t = []
                    for ft_ in range(FBT):
                        w = w1_p.tile([128, KD * 128], F32R,
                                      name=f"l{l}fb{fb}w1_{ft_}", tag="w1")
                        nc.sync.dma_start(w, w1_d[l, fb * FBT + ft_])
                        w1t.append(w)
                        w_ = w2_p.tile([128, D], F32R,
                                       name=f"l{l}fb{fb}w2_{ft_}", tag="w2")
                        nc.sync.dma_start(w_, w2_d[l, fb * FBT + ft_])
                        w2t.append(w_)
                    for th in range(2):
                        tcol = slice(th * 512, (th + 1) * 512)
                        uts = []
                        for ft_ in range(FBT):
                            ps = fpsum.tile([128, 512], F32, name="ups", tag="fps")
                            _mm_accum(nc, ps,
                                      [(w1t[ft_][:, k * 128 : (k + 1) * 128],
                                        h2fm[k][:, tcol]) for k in range(KD)],
                                      f32r=True)
                            ut = u_p.tile([128, 512], F32R,
                                          name=f"u{fb}_{th}_{ft_}", tag="u")
                            nc.scalar.activation(
                                ut, ps, ACT.Gelu_apprx_tanh,
                                bias=b1t[:, fb * FBT + ft_ : fb * FBT + ft_ + 1])
                            uts.append(ut)
                        for m in range(4):
                            xi = xt[th * 4 + m]
                            for n in range(2):
                                ps = fpsum.tile([128, 512], F32, name="yps", tag="fps")
                                _mm_accum(nc, ps,
                                          [(uts[kf][:, m * 128 : (m + 1) * 128],
                                            w2t[kf][:, n * 512 : (n + 1) * 512])
                                           for kf in range(FBT)], f32r=True)
                                nc.vector.tensor_tensor(
                                    xi[:, n * 512 : (n + 1) * 512],
                                    xi[:, n * 512 : (n + 1) * 512], ps, OP.add)
                if nonzero_b2:
                    b2_t = bias2_p.tile([128, D], F32, name=f"l{l}_b2bc")
                    nc.sync.dma_start(b2_t, b2_d[l])
                    for i in range(BL):
                        nc.vector.tensor_tensor(xt[i], xt[i], b2_t, OP.add)

        for i in range(BL):
            nc.sync.dma_start(y_out[i * 128 : (i + 1) * 128, :], xt[i])

    _split_multi_waits(nc)
    return nc


def prepare_host(inputs, n_layers=L):
    """Fold LN affines + biases into weights; arrange DMA-friendly layouts."""
    f32 = np.float32
    bf16 = ml_dtypes.bfloat16
    Wq = np.asarray(inputs["Wq"], f32)
    Wk = np.asarray(inputs["Wk"], f32)
    Wv = np.asarray(inputs["Wv"], f32)
    Wo = np.asarray(inputs["Wo"], f32)
    W1 = np.asarray(inputs["W1"], f32)
    W2 = np.asarray(inputs["W2"], f32)
    bq = np.asarray(inputs["bq"], f32)
    bk = np.asarray(inputs["bk"], f32)   # dropped: softmax row-shift invariance
    bv = np.asarray(inputs["bv"], f32)
    bo = np.asarray(inputs["bo"], f32)
    b1 = np.asarray(inputs["b1"], f32)
    b2 = np.asarray(inputs["b2"], f32)
    g1 = np.asarray(inputs["ln1_g"], f32)
    be1 = np.asarray(inputs["ln1_b"], f32)
    g2 = np.asarray(inputs["ln2_g"], f32)
    be2 = np.asarray(inputs["ln2_b"], f32)

    scale = np.float32(1.0 / np.sqrt(DK))
    Wq_e = (g1[:, :, None] * Wq) * scale
    bq_e = (bq + np.einsum("ld,ldo->lo", be1, Wq)) * scale
    Wk_e = Wk.copy()
    Wv_e = Wv.copy()
    bv_e = bv.copy()
    for l in range(1, L):
        Wk_e[l] = g1[l][:, None] * Wk[l]
        Wv_e[l] = g1[l][:, None] * Wv[l]
        bv_e[l] = bv[l] + be1[l] @ Wv[l]
    bo_e = bo + np.einsum("ld,ldo->lo", bv_e, Wo)
    W1_e = g2[:, :, None] * W1
    b1_e = b1 + np.einsum("ld,ldo->lo", be2, W1)

    def colblocks(w, nt):  # [L, D_in, N] -> [L, N/128, 128, (D_in/128)*128]
        kd = w.shape[1] // 128
        return np.ascontiguousarray(
            w.reshape(L, kd, 128, nt, 128).transpose(0, 3, 2, 1, 4)
        ).reshape(L, nt, 128, kd * 128)

    host = {
        "wq": colblocks(Wq_e, MD).astype(bf16),
        "wk": colblocks(Wk_e, MD).astype(bf16),
        "wv": np.ascontiguousarray(Wv_e.reshape(L, KD, 128, D)).astype(bf16),
        "wo": np.ascontiguousarray(Wo.reshape(L, KD, 128, D)).astype(bf16),
        "w1": colblocks(W1_e, FT).astype(f32),
        "w2": np.ascontiguousarray(W2.reshape(L, FT, 128, D)).astype(f32),
        "bq": np.ascontiguousarray(bq_e.reshape(L, MD, 128).transpose(0, 2, 1)),
        "b1": np.ascontiguousarray(b1_e.reshape(L, FT, 128).transpose(0, 2, 1)),
    }
    nonzero_bo = bool(np.any(bo_e))
    nonzero_b2 = bool(np.any(b2))
    if nonzero_bo:
        host["bo_bc"] = np.ascontiguousarray(
            np.broadcast_to(bo_e[:, None, :], (L, 128, D)).astype(f32))
    if nonzero_b2:
        host["b2_bc"] = np.ascontiguousarray(
            np.broadcast_to(b2[:, None, :], (L, 128, D)).astype(f32))

    xt = np.asarray(inputs["xt"], f32)
    p_att = np.asarray(inputs["p_att_feats"], f32)
    per_core = []
    for c in range(NCORES):
        xs = np.ascontiguousarray(xt[c * BL : (c + 1) * BL].reshape(T, D))
        kv = np.ascontiguousarray(
            p_att[c * BL : (c + 1) * BL].transpose(2, 0, 1).reshape(KD, 128, TC0)
        ).astype(bf16)
        m = dict(host)
        m["x"] = xs
        m["kv0"] = kv
        per_core.append(m)
    return per_core, nonzero_bo, nonzero_b2


def run(inputs, n_layers=L, trace=False, trace_dir=None):
    per_core, nz_bo, nz_b2 = prepare_host(inputs, n_layers)
    nc = build_program(nz_bo, nz_b2, n_layers)
    res = run_bass_kernel_spmd(nc, per_core, list(range(NCORES)))
    out = np.empty((B, S, D), np.float32)
    for c in range(NCORES):
        out[c * BL : (c + 1) * BL] = res.results[c]["y"].reshape(BL, S, D)
    return out


def kernel(**inputs) -> np.ndarray:
    return run(inputs)



# revision 30
# speedup vs baseline: 1.3166x; 1.3166x over previous
"""Trainium2 Bass kernel for nn_BertAoA_Decoder_Core (6-layer BERT-style encoder,
layer-0 cross-attention to p_att_feats).

Strategy: pure data-parallel over batch across 8 NeuronCores (8 examples/core).
All activations stay SBUF-resident for the whole 6-layer stack; weights stream
from HBM under PE compute.  Host-side algebraic folding removes LN affine params
and most biases.

v2: Q/K/V/O projections run in fp8-e4m3 with DoubleRow perf mode (2 contraction
rows per PE cell per cycle); weights are scaled host-side into e4m3 range
(TRN e4m3 max is +-240) and descaled at psum eviction.  FFN runs in bf16
(fp8 FFN fails the accuracy budget), which enables Fast Weight Load on the
PE weight path and halves FFN weight DMA vs f32r.  Attention smalls
(scores/softmax/AV) stay bf16.
"""

import sys

sys.path.insert(0, "/opt/trn_rl_repo")

import numpy as np
import ml_dtypes
from contextlib import ExitStack

import concourse.bass as bass
import concourse.mybir as mybir
import concourse.tile as tile
from concourse.masks import make_identity
from concourse.bass_utils import run_bass_kernel_spmd

F32 = mybir.dt.float32
BF16 = mybir.dt.bfloat16
F8 = mybir.dt.float8e4
I32 = mybir.dt.int32
AX = mybir.AxisListType.X
OP = mybir.AluOpType
ACT = mybir.ActivationFunctionType
DR = mybir.MatmulPerfMode.DoubleRow

# Problem constants (hardcoded per contract)
B, S, C, D, H, L, F = 64, 128, 196, 1024, 16, 6, 4096
DK = D // H              # 64
NCORES = 8
BL = B // NCORES         # 8 examples per core
T = BL * S               # 1024 query tokens per core
TC0 = BL * C             # 1568 context tokens per core (layer 0)
KD = D // 128            # 8 contraction tiles
KP = KD // 2             # 4 contraction pair-tiles (DoubleRow)
MD = D // 128            # 8 output tiles
FT = F // 128            # 32 FFN tiles
NFB = 4                  # FFN f-blocks
FBT = FT // NFB          # 8 f-tiles per block
NGRP = 2                 # example groups per core
GE = BL // NGRP          # 4 examples per group
GT = GE * S              # 512 tokens per group
GC = GE * C              # 784 context tokens per group (layer 0)
LN_EPS = 1e-6

# feature toggles (HW bring-up bisect; env-overridable)
import os as _os
USE_ACCUM_EXP = _os.environ.get("K_ACCUM_EXP", "0") == "1"    # activation accum_out row-sums
USE_POOL_NORM = _os.environ.get("K_POOL_NORM", "1") == "1"    # softmax normalize on GpSimd
USE_PACKED_APS = _os.environ.get("K_PACKED_APS", "1") == "1"  # 4-head AV psum packing


def _split_multi_waits(nc):
    """This container's walrus accepts only one sync-wait per CTRL instruction;
    hoist extra waits onto preceding NoOps on the same engine."""
    cnt = 0
    for fn in nc.m.functions:
        for bb in fn.blocks:
            new_list = []
            for ins in bb.instructions:
                si = getattr(ins, "sync_info", None)
                ow = getattr(si, "on_wait", None) if si is not None else None
                if ow and len(ow) > 1:
                    for w in ow[:-1]:
                        nop = mybir.InstNoOp(
                            name=f"{ins.name}-wsplit-{cnt}",
                            engine=ins.engine,
                            sync_info=mybir.SyncInfo(on_wait=[w], on_update=[]),
                        )
                        cnt += 1
                        new_list.append(nop)
                    si.on_wait = [ow[-1]]
                new_list.append(ins)
            bb.instructions = new_list
    return cnt


def _rsqrt(nc, pool, v_ap, out_ap, n):
    """out = 1/sqrt(v): exact DVE reciprocal + scalar-engine Sqrt (the
    recommended accurate pair; scalar Rsqrt/Reciprocal alone are inaccurate)."""
    r = pool.tile([128, n], F32, name="rs_r", tag="rs_r")
    nc.vector.reciprocal(r, v_ap)
    nc.scalar.activation(out_ap, r, ACT.Sqrt)


def _layer_norm(nc, stats_pool, x_tiles, h_pool, out_dtype, tagpfx):
    """Pre-norm (x-mu)*rstd for 8 [128, D] token-major tiles (affine folded
    into the weights host-side).  Processed in two half-batches so the second
    half's stats can start before the first half's consumers finish."""
    h_tiles = [None] * BL
    for hb in range(2):
        i0 = hb * (BL // 2)
        nb = BL // 2
        stat = stats_pool.tile([128, nb, 12], F32, name=f"{tagpfx}_stat{hb}",
                               tag=f"{tagpfx}_stat")
        mv = stats_pool.tile([128, nb, 2], F32, name=f"{tagpfx}_mv{hb}",
                             tag=f"{tagpfx}_mv")
        var = stats_pool.tile([128, nb], F32, name=f"{tagpfx}_var{hb}",
                              tag=f"{tagpfx}_var")
        rst = stats_pool.tile([128, nb], F32, name=f"{tagpfx}_rst{hb}",
                              tag=f"{tagpfx}_rst")
        for i in range(nb):
            nc.vector.bn_stats(stat[:, i, 0:6], x_tiles[i0 + i][:, 0:512])
            nc.vector.bn_stats(stat[:, i, 6:12], x_tiles[i0 + i][:, 512:1024])
            nc.vector.bn_aggr(mv[:, i, :], stat[:, i, :])
        nc.vector.tensor_scalar(var, mv[:, :, 1], LN_EPS, None, OP.add)
        _rsqrt(nc, stats_pool, var, rst, nb)
        for i in range(nb):
            h = h_pool.tile([128, D], out_dtype, name=f"{tagpfx}_h{i0+i}",
                            tag=f"{tagpfx}_h")
            nc.vector.tensor_scalar(h, x_tiles[i0 + i], mv[:, i, 0:1],
                                    rst[:, i : i + 1], OP.subtract, OP.mult)
            h_tiles[i0 + i] = h
    return h_tiles


def _transpose_quads(nc, tpsum, h_tm, ident, dst_writes, tag):
    """Token-major tiles [128, D] -> feature-major via [128,128] PE transposes
    packed four-at-a-time into one [128,512] psum; dst_writes(k, i0, ps) evicts."""
    for i0 in range(0, BL, 4):
        for k in range(KD):
            ps = tpsum.tile([128, 512], BF16, name=f"{tag}_tp4", tag="tp4")
            for i in range(i0, i0 + 4):
                nc.tensor.transpose(ps[:, (i - i0) * 128 : (i - i0 + 1) * 128],
                                    h_tm[i][:, k * 128 : (k + 1) * 128], ident)
            dst_writes(k, i0, ps)


def _mm_accum(nc, ps, pairs, perf_mode=None):
    n = len(pairs)
    for i, (lhsT, rhs) in enumerate(pairs):
        nc.tensor.matmul(ps, lhsT, rhs, start=(i == 0), stop=(i == n - 1),
                         perf_mode=perf_mode)


def build_program(nonzero_bo, nonzero_b2, inv_s, n_layers=L, split_waits=True):
    """inv_s: dict of per-layer descale lists: q, k, v, o (floats)."""
    nc = bass.Bass()
    x_in = nc.declare_dram_parameter("x", [T, D], F32, isOutput=False)
    y_out = nc.declare_dram_parameter("y", [T, D], F32, isOutput=True)
    kv0 = nc.declare_dram_parameter("kv0", [KP, 128, 2, TC0], F8, isOutput=False)
    wq_d = nc.declare_dram_parameter("wq", [L, MD, 128, KP, 2, 128], F8, isOutput=False)
    wk_d = nc.declare_dram_parameter("wk", [L, MD, 128, KP, 2, 128], F8, isOutput=False)
    wv_d = nc.declare_dram_parameter("wv", [L, KP, 128, 2, D], F8, isOutput=False)
    wo_d = nc.declare_dram_parameter("wo", [L, KP, 128, 2, D], F8, isOutput=False)
    w1_d = nc.declare_dram_parameter("w1", [L, FT, 128, KD * 128], BF16, isOutput=False)
    w2_d = nc.declare_dram_parameter("w2", [L, FT, 128, D], BF16, isOutput=False)
    bq_d = nc.declare_dram_parameter("bq", [L, 128, MD], F32, isOutput=False)
    b1_d = nc.declare_dram_parameter("b1", [L, 128, FT], F32, isOutput=False)
    if nonzero_bo:
        bo_d = nc.declare_dram_parameter("bo_bc", [L, 128, D], F32, isOutput=False)
    if nonzero_b2:
        b2_d = nc.declare_dram_parameter("b2_bc", [L, 128, D], F32, isOutput=False)

    with tile.TileContext(nc) as tc, ExitStack() as top:
        const = top.enter_context(tc.tile_pool(name="const", bufs=1))
        ident_bf = const.tile([128, 128], BF16, name="ident_bf")
        make_identity(nc, ident_bf)

        xpool = top.enter_context(tc.tile_pool(name="xres", bufs=BL))
        xt = []
        for i in range(BL):
            t_ = xpool.tile([128, D], F32, name=f"x{i}", tag="x")
            nc.sync.dma_start(t_, x_in[i * 128 : (i + 1) * 128, :])
            xt.append(t_)

        for l in range(n_layers):
            isq, isk, isv, iso = (inv_s[c][l] for c in "qkvo")
            # ---------------- attention sublayer ----------------
            with ExitStack() as actx:
                stats = actx.enter_context(tc.tile_pool(name=f"l{l}_st", bufs=1))
                h1tm_p = actx.enter_context(tc.tile_pool(name=f"l{l}_h1tm", bufs=4))
                h1fm_p = actx.enter_context(tc.tile_pool(name=f"l{l}_h1fm", bufs=1))
                wqk_p = actx.enter_context(tc.tile_pool(name=f"l{l}_wqk", bufs=16))
                wvo_p = actx.enter_context(tc.tile_pool(name=f"l{l}_wvo", bufs=8))
                gbufs = 1 if l == 0 else 2
                qa_p = actx.enter_context(tc.tile_pool(name=f"l{l}_qa", bufs=gbufs))
                kg_p = actx.enter_context(tc.tile_pool(name=f"l{l}_kg", bufs=gbufs))
                v_p = actx.enter_context(tc.tile_pool(name=f"l{l}_v", bufs=8))
                ag_p = actx.enter_context(tc.tile_pool(name=f"l{l}_ag", bufs=gbufs))
                sm_p = actx.enter_context(tc.tile_pool(name=f"l{l}_sm", bufs=4))
                sc_p = actx.enter_context(tc.tile_pool(name=f"l{l}_sc", bufs=4))
                ot_p = actx.enter_context(tc.tile_pool(name=f"l{l}_ot", bufs=3))
                bias_p = actx.enter_context(tc.tile_pool(name=f"l{l}_bias", bufs=1))
                ppsum = actx.enter_context(
                    tc.tile_pool(name=f"l{l}_pps", bufs=2, space="PSUM"))
                spsum = actx.enter_context(
                    tc.tile_pool(name=f"l{l}_sps", bufs=2 if l == 0 else 3,
                                 space="PSUM"))
                tpsum = actx.enter_context(
                    tc.tile_pool(name=f"l{l}_tps", bufs=2, space="PSUM"))
                apsum = actx.enter_context(
                    tc.tile_pool(name=f"l{l}_aps", bufs=2 if l == 0 else 1,
                                 space="PSUM"))
                if l == 0:
                    kv0_p = actx.enter_context(tc.tile_pool(name="l0_kv0", bufs=KP))

                bqt = bias_p.tile([128, MD], F32, name=f"l{l}_bqt")
                nc.sync.dma_start(bqt, bq_d[l])

                # LayerNorm 1 (token-major bf16) -> transpose -> fp8 pair tile
                h1tm = _layer_norm(nc, stats, xt, h1tm_p, BF16, f"l{l}a")
                hf1 = h1fm_p.tile([128, KP, 2, T], F8, name=f"l{l}_hf1", tag="hf1")

                def _hf1_write(k, i0, ps):
                    nc.vector.tensor_copy(
                        hf1[:, k // 2, k % 2, i0 * 128 : (i0 + 4) * 128], ps)

                _transpose_quads(nc, tpsum, h1tm, ident_bf, _hf1_write, f"l{l}a")

                # weight tiles for the whole layer (fp8, DoubleRow pair layout)
                wq_t = []
                wk_t = []
                for m in range(MD):
                    w = wqk_p.tile([128, KP, 2, 128], F8, name=f"l{l}_wq{m}", tag="wqk")
                    nc.sync.dma_start(w, wq_d[l, m])
                    wq_t.append(w)
                for m in range(MD):
                    w = wqk_p.tile([128, KP, 2, 128], F8, name=f"l{l}_wk{m}", tag="wqk")
                    nc.sync.dma_start(w, wk_d[l, m])
                    wk_t.append(w)
                wv_t = []
                wo_t = []
                for k in range(KP):
                    w = wvo_p.tile([128, 2, D], F8, name=f"l{l}_wv{k}", tag="wvo")
                    nc.sync.dma_start(w, wv_d[l, k])
                    wv_t.append(w)
                for k in range(KP):
                    w = wvo_p.tile([128, 2, D], F8, name=f"l{l}_wo{k}", tag="wvo")
                    nc.sync.dma_start(w, wo_d[l, k])
                    wo_t.append(w)

                TCB = C if l == 0 else S          # context length per example
                TCG = GE * TCB                    # per group

                # ---- per-group state ----
                qg_t = [None] * NGRP
                kg_t = [None] * NGRP
                ag_t = [None] * NGRP
                vts_t = [None] * NGRP

                def emit_qkv(g):
                    """Generator of per-psum emit closures for Q/K/V of group g."""
                    gcol = slice(g * GT, (g + 1) * GT)
                    if l == 0:
                        kvg = []
                        for k in range(KP):
                            kt = kv0_p.tile([128, 2, GC], F8, name=f"kv0_{k}",
                                            tag="kv0")
                            nc.sync.dma_start(kt, kv0[k, :, :, g * GC : (g + 1) * GC])
                            kvg.append(kt)
                    else:
                        kvg = None

                    qg = qa_p.tile([128, MD, GT], BF16, name=f"l{l}g{g}_q", tag="qg")
                    kg = kg_p.tile([128, MD, TCG], BF16, name=f"l{l}g{g}_k", tag="kg")
                    qg_t[g] = qg
                    kg_t[g] = kg
                    closures = []
                    for m in range(MD):
                        def _q(m=m):
                            ps = ppsum.tile([128, 512], F32, name="qps", tag="pps")
                            _mm_accum(nc, ps,
                                      [(wq_t[m][:, k], hf1[:, k, :, gcol])
                                       for k in range(KP)], DR)
                            nc.scalar.activation(qg[:, m, :], ps, ACT.Identity,
                                                 bias=bqt[:, m : m + 1], scale=isq)
                        closures.append(_q)
                    for m in range(MD):
                        for n0 in range(0, TCG, 512):
                            def _k(m=m, n0=n0):
                                n1 = min(n0 + 512, TCG)
                                ps = ppsum.tile([128, 512], F32, name="kps", tag="pps")
                                if l == 0:
                                    rh = [(wk_t[m][:, k], kvg[k][:, :, n0:n1])
                                          for k in range(KP)]
                                else:
                                    rh = [(wk_t[m][:, k],
                                           hf1[:, k, :, g * GT + n0 : g * GT + n1])
                                          for k in range(KP)]
                                _mm_accum(nc, ps[:, : n1 - n0], rh, DR)
                                nc.vector.tensor_scalar(kg[:, m, n0:n1],
                                                        ps[:, : n1 - n0],
                                                        isk, None, OP.mult)
                            closures.append(_k)
                    vts = []
                    for e in range(GE):
                        segs = []
                        for s0 in range(0, TCB, 128):
                            nrows = min(128, TCB - s0)
                            vt = v_p.tile([128, D], BF16, name=f"l{l}g{g}e{e}v{s0}",
                                          tag="v")
                            segs.append((vt, nrows))
                        vts.append(segs)
                    vts_t[g] = vts
                    for e in range(GE):
                        for si, (vt, nrows) in enumerate(vts[e]):
                            s0 = si * 128
                            for n in range(2):
                                def _v(e=e, s0=s0, nrows=nrows, vt=vt, n=n):
                                    ps = ppsum.tile([128, 512], F32, name="vps",
                                                    tag="pps")
                                    if l == 0:
                                        lh = [(kvg[k][:, :, e * TCB + s0 :
                                                       e * TCB + s0 + nrows],
                                               wv_t[k][:, :, n * 512 : (n + 1) * 512])
                                              for k in range(KP)]
                                    else:
                                        c0 = (g * GE + e) * 128
                                        lh = [(hf1[:, k, :, c0 : c0 + 128],
                                               wv_t[k][:, :, n * 512 : (n + 1) * 512])
                                              for k in range(KP)]
                                    _mm_accum(nc, ps[: nrows], lh, DR)
                                    nc.vector.tensor_scalar(
                                        vt[:nrows, n * 512 : (n + 1) * 512],
                                        ps[:nrows], isv, None, OP.mult)
                                closures.append(_v)
                    return closures

                def emit_o(g):
                    """Per-psum closures for output projection + residual."""
                    ag = ag_t[g]
                    closures = []
                    for e in range(GE):
                        xi = xt[g * GE + e]
                        for n in range(2):
                            def _o(e=e, xi=xi, n=n):
                                ps = ppsum.tile([128, 512], F32, name="ops", tag="pps")
                                _mm_accum(nc, ps,
                                          [(ag[:, k, :, e * 128 : (e + 1) * 128],
                                            wo_t[k][:, :, n * 512 : (n + 1) * 512])
                                           for k in range(KP)], DR)
                                xs = xi[:, n * 512 : (n + 1) * 512]
                                ot = ot_p.tile([128, 512], F32, name="ot", tag="ot")
                                nc.scalar.activation(ot, ps, ACT.Copy, scale=iso)
                                nc.gpsimd.tensor_tensor(xs, xs, ot, OP.add)
                            closures.append(_o)
                    return closures

                # ---- attention smalls: software-pipelined blocks ----
                # Per block (example e, bh consecutive heads):
                #   front: per-head scores matmul (K=64, offset-0 psum) ->
                #          exp on ACT with accum_out row-sums -> recip (DVE)
                #          -> normalize (Pool, free-dim-broadcast rinv)
                #   back:  p-transposes (K=128, packed psum) -> pts copy (DVE)
                #          -> AV matmuls -> ag eviction (ACT, fp8 cast)
                # Blocks run SKEW apart so PE never waits on the softmax chain;
                # projection psums of the other group interleave between blocks.
                bh = 512 // TCB          # 4 heads (S=128) or 2 heads (C=196)
                nseg = (TCB + 127) // 128
                SKEW = 2

                def smalls_front(g, e, hb):
                    qg, kg = qg_t[g], kg_t[g]
                    praw = sm_p.tile([128, bh, TCB], BF16, name="praw", tag="praw")
                    ssum = sc_p.tile([128, bh], F32, name="ssum", tag="ssum")
                    for hi in range(bh):
                        h_ = hb + hi
                        po, ch = 64 * (h_ % 2), h_ // 2
                        sp = spsum.tile([128, TCB], F32, name="sp", tag="sps")
                        nc.tensor.matmul(
                            sp,
                            qg[po : po + 64, ch, e * 128 : (e + 1) * 128],
                            kg[po : po + 64, ch, e * TCB : (e + 1) * TCB],
                            start=True, stop=True)
                        # exp without max-subtraction (|scores| << 80); row sums
                        # accumulate on the fly
                        if USE_ACCUM_EXP:
                            nc.scalar.activation(praw[:, hi, :], sp, ACT.Exp,
                                                 accum_out=ssum[:, hi : hi + 1])
                        else:
                            nc.scalar.activation(praw[:, hi, :], sp, ACT.Exp)
                    if not USE_ACCUM_EXP:
                        nc.vector.tensor_reduce(ssum, praw, AX, OP.add)
                    rinv = sc_p.tile([128, bh], F32, name="rinv", tag="rinv")
                    nc.vector.reciprocal(rinv, ssum)
                    pbf = sm_p.tile([128, bh, TCB], BF16, name="pbf", tag="pbf")
                    eng = nc.gpsimd if USE_POOL_NORM else nc.vector
                    eng.tensor_tensor(
                        pbf, praw,
                        rinv[:, :, None].broadcast_to((128, bh, TCB)),
                        OP.mult)
                    return pbf

                def smalls_back1(g, e, hb, pbf):
                    tp4 = tpsum.tile([128, nseg, bh, 128], BF16,
                                     name="ptp4", tag="tp4")
                    for hi in range(bh):
                        for si in range(nseg):
                            nrows = min(128, TCB - si * 128)
                            nc.tensor.transpose(
                                tp4[:nrows, si, hi, :],
                                pbf[:, hi, si * 128 : si * 128 + nrows],
                                ident_bf)
                    pts = sm_p.tile([128, nseg, bh, 128], BF16,
                                    name="pts", tag="pts")
                    for si in range(nseg):
                        nrows = min(128, TCB - si * 128)
                        nc.vector.tensor_copy(pts[:nrows, si], tp4[:nrows, si])
                    return pts

                def smalls_back2(g, e, hb, pts):
                    ag, vts = ag_t[g], vts_t[g]
                    if l > 0 and bh == 4 and USE_PACKED_APS:
                        # packed AV psum: 4 heads in one bank, all at partition
                        # offset 0 (K=128 col-offset writes are safe; partition
                        # -offset matmul writes hard-fault this silicon).
                        # Slot order groups the po=0 heads (hb, hb+2) in slots
                        # 0:2 and po=64 heads in slots 2:4 so each ag half
                        # evicts as one contiguous [64, 2, 128] activation.
                        aps = apsum.tile([64, 4, 128], F32, name="aps", tag="aps")
                        for hi in range(bh):
                            h_ = hb + hi
                            slot = (hi % 2) * 2 + hi // 2
                            vt, nrows = vts[e][0]
                            nc.tensor.matmul(
                                aps[:, slot, :],
                                vt[:nrows, h_ * 64 : (h_ + 1) * 64],
                                pts[:nrows, 0, hi, :],
                                start=True, stop=True)
                        ecol = slice(e * 128, (e + 1) * 128)
                        nc.scalar.activation(
                            ag[0:64, hb // 4, :, ecol], aps[:, 0:2, :], ACT.Copy)
                        nc.scalar.activation(
                            ag[64:128, hb // 4, :, ecol], aps[:, 2:4, :], ACT.Copy)
                    else:
                        for hi in range(bh):
                            h_ = hb + hi
                            po, ch = 64 * (h_ % 2), h_ // 2
                            aps = apsum.tile([64, 128], F32, name="aps0", tag="aps")
                            for si in range(nseg):
                                nrows = min(128, TCB - si * 128)
                                vt, _ = vts[e][si]
                                nc.tensor.matmul(
                                    aps, vt[:nrows, h_ * 64 : (h_ + 1) * 64],
                                    pts[:nrows, si, hi, :],
                                    start=(si == 0), stop=(si == nseg - 1))
                            nc.scalar.activation(
                                ag[po : po + 64, ch // 2, ch % 2,
                                   e * 128 : (e + 1) * 128],
                                aps, ACT.Copy)

                def smalls_group(g, fillers):
                    """Pipelined smalls for group g, interleaving `fillers`
                    (projection psum closures) between blocks.  Transposes run
                    SKEW slots behind scores; AV one more slot behind so the
                    pts SBUF copy has a full slot to land."""
                    ag_t[g] = ag_p.tile([128, KP, 2, GT], F8,
                                        name=f"l{l}g{g}_a", tag="ag")
                    blocks = [(e, hb) for e in range(GE) for hb in range(0, H, bh)]
                    fq = list(fillers)
                    fi = 0
                    nfill = len(fq)
                    nb = len(blocks)
                    nslots = nb + SKEW + 1
                    pend1 = []
                    pend2 = []
                    for bi in range(nslots):
                        if bi < nb:
                            e, hb = blocks[bi]
                            pend1.append((e, hb, smalls_front(g, e, hb)))
                        want = nfill * (bi + 1) // nslots
                        while fi < want:
                            fq[fi]()
                            fi += 1
                        if bi >= SKEW and pend1:
                            e, hb, pbf = pend1.pop(0)
                            pend2.append((e, hb, smalls_back1(g, e, hb, pbf)))
                        if bi >= SKEW + 1 and pend2:
                            e, hb, pts = pend2.pop(0)
                            smalls_back2(g, e, hb, pts)

                # ---- layer schedule ----
                qkv0 = emit_qkv(0)
                for c in qkv0:
                    c()
                qkv1 = emit_qkv(1)
                smalls_group(0, qkv1)
                smalls_group(1, emit_o(0))
                for c in emit_o(1):
                    c()
                if nonzero_bo:
                    bo_t = bias_p.tile([128, D], F32, name=f"l{l}_bo")
                    nc.sync.dma_start(bo_t, bo_d[l])
                    for i in range(BL):
                        nc.vector.tensor_tensor(xt[i], xt[i], bo_t, OP.add)

            # ---------------- FFN sublayer ----------------
            with ExitStack() as fctx:
                stats2 = fctx.enter_context(tc.tile_pool(name=f"l{l}_st2", bufs=1))
                h2tm_p = fctx.enter_context(tc.tile_pool(name=f"l{l}_h2tm", bufs=4))
                h2fm_p = fctx.enter_context(tc.tile_pool(name=f"l{l}_h2fm", bufs=KD))
                w1_p = fctx.enter_context(tc.tile_pool(name=f"l{l}_w1", bufs=12))
                w2_p = fctx.enter_context(tc.tile_pool(name=f"l{l}_w2", bufs=12))
                u_p = fctx.enter_context(tc.tile_pool(name=f"l{l}_u", bufs=12))
                bias2_p = fctx.enter_context(tc.tile_pool(name=f"l{l}_b2", bufs=1))
                fpsum = fctx.enter_context(
                    tc.tile_pool(name=f"l{l}_fps", bufs=4, space="PSUM"))
                tpsum2 = fctx.enter_context(
                    tc.tile_pool(name=f"l{l}_tps2", bufs=2, space="PSUM"))

                b1t = bias2_p.tile([128, FT], F32, name=f"l{l}_b1t")
                nc.sync.dma_start(b1t, b1_d[l])

                h2tm = _layer_norm(nc, stats2, xt, h2tm_p, BF16, f"l{l}f")
                h2fm = [h2fm_p.tile([128, T], BF16, name=f"l{l}f_fm{k}", tag="h2fm")
                        for k in range(KD)]

                def _h2_write(k, i0, ps):
                    nc.vector.tensor_copy(h2fm[k][:, i0 * 128 : (i0 + 4) * 128], ps)

                _transpose_quads(nc, tpsum2, h2tm, ident_bf, _h2_write, f"l{l}f")

                for fb in range(NFB):
                    w1t = []
                    w2t = []
                    for ft_ in range(FBT):
                        w = w1_p.tile([128, KD * 128], BF16,
                                      name=f"l{l}fb{fb}w1_{ft_}", tag="w1")
                        nc.sync.dma_start(w, w1_d[l, fb * FBT + ft_])
                        w1t.append(w)
                        w_ = w2_p.tile([128, D], BF16,
                                       name=f"l{l}fb{fb}w2_{ft_}", tag="w2")
                        nc.sync.dma_start(w_, w2_d[l, fb * FBT + ft_])
                        w2t.append(w_)
                    for th in range(2):
                        tcol = slice(th * 512, (th + 1) * 512)
                        uts = []
                        for ft_ in range(FBT):
                            ps = fpsum.tile([128, 512], F32, name="ups", tag="fps")
                            _mm_accum(nc, ps,
                                      [(w1t[ft_][:, k * 128 : (k + 1) * 128],
                                        h2fm[k][:, tcol]) for k in range(KD)])
                            ut = u_p.tile([128, 512], BF16,
                                          name=f"u{fb}_{th}_{ft_}", tag="u")
                            nc.scalar.activation(
                                ut, ps, ACT.Gelu_apprx_tanh,
                                bias=b1t[:, fb * FBT + ft_ : fb * FBT + ft_ + 1])
                            uts.append(ut)
                        for m in range(4):
                            xi = xt[th * 4 + m]
                            for n in range(2):
                                ps = fpsum.tile([128, 512], F32, name="yps", tag="fps")
                                _mm_accum(nc, ps,
                                          [(uts[kf][:, m * 128 : (m + 1) * 128],
                                            w2t[kf][:, n * 512 : (n + 1) * 512])
                                           for kf in range(FBT)])
                                xs = xi[:, n * 512 : (n + 1) * 512]
                                if fb == NFB - 1 and th == 1:
                                    # keep the DVE FIFO clear at the layer
                                    # boundary: next layer's LN1 stats must not
                                    # queue behind these evictions
                                    ot = u_p.tile([128, 512], F32, name="fot",
                                                  tag="fot")
                                    nc.scalar.activation(ot, ps, ACT.Copy)
                                    nc.gpsimd.tensor_tensor(xs, xs, ot, OP.add)
                                else:
                                    nc.vector.tensor_tensor(xs, xs, ps, OP.add)
                if nonzero_b2:
                    b2_t = bias2_p.tile([128, D], F32, name=f"l{l}_b2bc")
                    nc.sync.dma_start(b2_t, b2_d[l])
                    for i in range(BL):
                        nc.vector.tensor_tensor(xt[i], xt[i], b2_t, OP.add)

        for i in range(BL):
            nc.sync.dma_start(y_out[i * 128 : (i + 1) * 128, :], xt[i])

    if split_waits:
        _split_multi_waits(nc)
    return nc


def _q8(w, s):
    """host-side e4m3 (TRN variant, max 240) quantization of w*s"""
    e4m3 = ml_dtypes.float8_e4m3
    return np.clip(np.asarray(w, np.float32) * s, -240.0, 240.0).astype(e4m3)


def prepare_host(inputs, n_layers=L):
    """Fold LN affines + biases into weights; arrange DMA-friendly layouts."""
    f32 = np.float32
    bf16 = ml_dtypes.bfloat16
    e4m3 = ml_dtypes.float8_e4m3
    Wq = np.asarray(inputs["Wq"], f32)
    Wk = np.asarray(inputs["Wk"], f32)
    Wv = np.asarray(inputs["Wv"], f32)
    Wo = np.asarray(inputs["Wo"], f32)
    W1 = np.asarray(inputs["W1"], f32)
    W2 = np.asarray(inputs["W2"], f32)
    bq = np.asarray(inputs["bq"], f32)
    bk = np.asarray(inputs["bk"], f32)   # dropped: softmax row-shift invariance
    bv = np.asarray(inputs["bv"], f32)
    bo = np.asarray(inputs["bo"], f32)
    b1 = np.asarray(inputs["b1"], f32)
    b2 = np.asarray(inputs["b2"], f32)
    g1 = np.asarray(inputs["ln1_g"], f32)
    be1 = np.asarray(inputs["ln1_b"], f32)
    g2 = np.asarray(inputs["ln2_g"], f32)
    be2 = np.asarray(inputs["ln2_b"], f32)

    scale = np.float32(1.0 / np.sqrt(DK))
    Wq_e = (g1[:, :, None] * Wq) * scale
    bq_e = (bq + np.einsum("ld,ldo->lo", be1, Wq)) * scale
    Wk_e = Wk.copy()
    Wv_e = Wv.copy()
    bv_e = bv.copy()
    for l in range(1, L):
        Wk_e[l] = g1[l][:, None] * Wk[l]
        Wv_e[l] = g1[l][:, None] * Wv[l]
        bv_e[l] = bv[l] + be1[l] @ Wv[l]
    bo_e = bo + np.einsum("ld,ldo->lo", bv_e, Wo)
    W1_e = g2[:, :, None] * W1
    b1_e = b1 + np.einsum("ld,ldo->lo", be2, W1)

    # per-layer fp8 weight scales (e4m3 abs-max target 192; TRN max 240)
    def _scales(w):
        return [np.float32(192.0 / max(np.abs(w[l]).max(), 1e-8))
                for l in range(L)]

    s_q, s_k, s_v, s_o = (_scales(w) for w in (Wq_e, Wk_e, Wv_e, Wo))
    inv_s = {
        "q": [float(1.0 / s) for s in s_q],
        "k": [float(1.0 / s) for s in s_k],
        "v": [float(1.0 / s) for s in s_v],
        "o": [float(1.0 / s) for s in s_o],
    }

    def colblocks(w):  # [L, D_in, N] -> [L, N/128, 128, (D_in/128)*128]
        kd = w.shape[1] // 128
        nt = w.shape[2] // 128
        return np.ascontiguousarray(
            w.reshape(L, kd, 128, nt, 128).transpose(0, 3, 2, 1, 4)
        ).reshape(L, nt, 128, kd * 128)

    def rowpairs(w):   # [L, D_in, N] -> [L, KP, 128, 2, N]
        return np.ascontiguousarray(
            w.reshape(L, KP, 2, 128, w.shape[2]).transpose(0, 1, 3, 2, 4))

    sq_a = np.asarray(s_q, f32)[:, None, None]
    sk_a = np.asarray(s_k, f32)[:, None, None]
    sv_a = np.asarray(s_v, f32)[:, None, None]
    so_a = np.asarray(s_o, f32)[:, None, None]

    host = {
        "wq": _q8(colblocks(Wq_e * sq_a), 1.0).reshape(L, MD, 128, KP, 2, 128),
        "wk": _q8(colblocks(Wk_e * sk_a), 1.0).reshape(L, MD, 128, KP, 2, 128),
        "wv": _q8(rowpairs(Wv_e * sv_a), 1.0),
        "wo": _q8(rowpairs(Wo * so_a), 1.0),
        "w1": colblocks(W1_e).astype(bf16),
        "w2": np.ascontiguousarray(W2.reshape(L, FT, 128, D)).astype(bf16),
        "bq": np.ascontiguousarray(bq_e.reshape(L, MD, 128).transpose(0, 2, 1)),
        "b1": np.ascontiguousarray(b1_e.reshape(L, FT, 128).transpose(0, 2, 1)),
    }
    nonzero_bo = bool(np.any(bo_e))
    nonzero_b2 = bool(np.any(b2))
    if nonzero_bo:
        host["bo_bc"] = np.ascontiguousarray(
            np.broadcast_to(bo_e[:, None, :], (L, 128, D)).astype(f32))
    if nonzero_b2:
        host["b2_bc"] = np.ascontiguousarray(
            np.broadcast_to(b2[:, None, :], (L, 128, D)).astype(f32))

    xt = np.asarray(inputs["xt"], f32)
    p_att = np.asarray(inputs["p_att_feats"], f32)
    per_core = []
    for c in range(NCORES):
        xs = np.ascontiguousarray(xt[c * BL : (c + 1) * BL].reshape(T, D))
        kv = p_att[c * BL : (c + 1) * BL].transpose(2, 0, 1).reshape(KD, 128, TC0)
        kv = np.ascontiguousarray(
            kv.reshape(KP, 2, 128, TC0).transpose(0, 2, 1, 3))
        m = dict(host)
        m["x"] = xs
        m["kv0"] = np.clip(kv, -240.0, 240.0).astype(e4m3)
        per_core.append(m)
    return per_core, nonzero_bo, nonzero_b2, inv_s


def run(inputs, n_layers=L, trace=False, trace_dir=None):
    per_core, nz_bo, nz_b2, inv_s = prepare_host(inputs, n_layers)
    nc = build_program(nz_bo, nz_b2, inv_s, n_layers)
    res = run_bass_kernel_spmd(nc, per_core, list(range(NCORES)))
    out = np.empty((B, S, D), np.float32)
    for c in range(NCORES):
        out[c * BL : (c + 1) * BL] = res.results[c]["y"].reshape(BL, S, D)
    return out


def kernel(**inputs) -> np.ndarray:
    return run(inputs)


# revision 32
# speedup vs baseline: 1.3178x; 1.0009x over previous
"""Trainium2 Bass kernel for nn_BertAoA_Decoder_Core (6-layer BERT-style encoder,
layer-0 cross-attention to p_att_feats).

Strategy: pure data-parallel over batch across 8 NeuronCores (8 examples/core).
All activations stay SBUF-resident for the whole 6-layer stack; weights stream
from HBM under PE compute.  Host-side algebraic folding removes LN affine params
and most biases.

v2: Q/K/V/O projections run in fp8-e4m3 with DoubleRow perf mode (2 contraction
rows per PE cell per cycle); weights are scaled host-side into e4m3 range
(TRN e4m3 max is +-240) and descaled at psum eviction.  FFN runs in bf16
(fp8 FFN fails the accuracy budget), which enables Fast Weight Load on the
PE weight path and halves FFN weight DMA vs f32r.  Attention smalls
(scores/softmax/AV) stay bf16.
"""

import sys

sys.path.insert(0, "/opt/trn_rl_repo")

import numpy as np
import ml_dtypes
from contextlib import ExitStack

import concourse.bass as bass
import concourse.mybir as mybir
import concourse.tile as tile
from concourse.masks import make_identity
from concourse.bass_utils import run_bass_kernel_spmd

F32 = mybir.dt.float32
BF16 = mybir.dt.bfloat16
F8 = mybir.dt.float8e4
I32 = mybir.dt.int32
AX = mybir.AxisListType.X
OP = mybir.AluOpType
ACT = mybir.ActivationFunctionType
DR = mybir.MatmulPerfMode.DoubleRow

# Problem constants (hardcoded per contract)
B, S, C, D, H, L, F = 64, 128, 196, 1024, 16, 6, 4096
DK = D // H              # 64
NCORES = 8
BL = B // NCORES         # 8 examples per core
T = BL * S               # 1024 query tokens per core
TC0 = BL * C             # 1568 context tokens per core (layer 0)
KD = D // 128            # 8 contraction tiles
KP = KD // 2             # 4 contraction pair-tiles (DoubleRow)
MD = D // 128            # 8 output tiles
FT = F // 128            # 32 FFN tiles
NFB = 4                  # FFN f-blocks
FBT = FT // NFB          # 8 f-tiles per block
NGRP = 2                 # example groups per core
GE = BL // NGRP          # 4 examples per group
GT = GE * S              # 512 tokens per group
GC = GE * C              # 784 context tokens per group (layer 0)
LN_EPS = 1e-6

# feature toggles (HW bring-up bisect; env-overridable)
import os as _os
USE_ACCUM_EXP = _os.environ.get("K_ACCUM_EXP", "0") == "1"    # activation accum_out row-sums
USE_POOL_NORM = _os.environ.get("K_POOL_NORM", "1") == "1"    # softmax normalize on GpSimd
USE_PACKED_APS = _os.environ.get("K_PACKED_APS", "1") == "1"  # 4-head AV psum packing


def _split_multi_waits(nc):
    """This container's walrus accepts only one sync-wait per CTRL instruction;
    hoist extra waits onto preceding NoOps on the same engine."""
    cnt = 0
    for fn in nc.m.functions:
        for bb in fn.blocks:
            new_list = []
            for ins in bb.instructions:
                si = getattr(ins, "sync_info", None)
                ow = getattr(si, "on_wait", None) if si is not None else None
                if ow and len(ow) > 1:
                    for w in ow[:-1]:
                        nop = mybir.InstNoOp(
                            name=f"{ins.name}-wsplit-{cnt}",
                            engine=ins.engine,
                            sync_info=mybir.SyncInfo(on_wait=[w], on_update=[]),
                        )
                        cnt += 1
                        new_list.append(nop)
                    si.on_wait = [ow[-1]]
                new_list.append(ins)
            bb.instructions = new_list
    return cnt


def _rsqrt(nc, pool, v_ap, out_ap, n):
    """out = 1/sqrt(v): exact DVE reciprocal + scalar-engine Sqrt (the
    recommended accurate pair; scalar Rsqrt/Reciprocal alone are inaccurate)."""
    r = pool.tile([128, n], F32, name="rs_r", tag="rs_r")
    nc.vector.reciprocal(r, v_ap)
    nc.scalar.activation(out_ap, r, ACT.Sqrt)


def _layer_norm(nc, stats_pool, x_tiles, h_pool, out_dtype, tagpfx):
    """Pre-norm (x-mu)*rstd for 8 [128, D] token-major tiles (affine folded
    into the weights host-side).  Processed in two half-batches so the second
    half's stats can start before the first half's consumers finish."""
    h_tiles = [None] * BL
    for hb in range(2):
        i0 = hb * (BL // 2)
        nb = BL // 2
        stat = stats_pool.tile([128, nb, 12], F32, name=f"{tagpfx}_stat{hb}",
                               tag=f"{tagpfx}_stat")
        mv = stats_pool.tile([128, nb, 2], F32, name=f"{tagpfx}_mv{hb}",
                             tag=f"{tagpfx}_mv")
        var = stats_pool.tile([128, nb], F32, name=f"{tagpfx}_var{hb}",
                              tag=f"{tagpfx}_var")
        rst = stats_pool.tile([128, nb], F32, name=f"{tagpfx}_rst{hb}",
                              tag=f"{tagpfx}_rst")
        for i in range(nb):
            nc.vector.bn_stats(stat[:, i, 0:6], x_tiles[i0 + i][:, 0:512])
            nc.vector.bn_stats(stat[:, i, 6:12], x_tiles[i0 + i][:, 512:1024])
            nc.vector.bn_aggr(mv[:, i, :], stat[:, i, :])
        nc.vector.tensor_scalar(var, mv[:, :, 1], LN_EPS, None, OP.add)
        _rsqrt(nc, stats_pool, var, rst, nb)
        # -mu*rstd biases for the ACT-side applies (per-partition AP)
        nmr = stats_pool.tile([128, nb], F32, name=f"{tagpfx}_nmr{hb}",
                              tag=f"{tagpfx}_nmr")
        nc.vector.tensor_tensor(nmr, mv[:, :, 0], rst, OP.mult)
        nc.vector.tensor_scalar(nmr, nmr, -1.0, None, OP.mult)
        for i in range(nb):
            h = h_pool.tile([128, D], out_dtype, name=f"{tagpfx}_h{i0+i}",
                            tag=f"{tagpfx}_h")
            if i % 2 == 0:
                # even tiles on DVE: (x - mu) * rstd
                nc.vector.tensor_scalar(h, x_tiles[i0 + i], mv[:, i, 0:1],
                                        rst[:, i : i + 1], OP.subtract, OP.mult)
            else:
                # odd tiles on ACT: x*rstd + (-mu*rstd) — halves the serial
                # apply chain that gates the first feature-major transposes
                nc.scalar.activation(h, x_tiles[i0 + i], ACT.Identity,
                                     bias=nmr[:, i : i + 1],
                                     scale=rst[:, i : i + 1])
            h_tiles[i0 + i] = h
    return h_tiles


def _transpose_quads(nc, tpsum, h_tm, ident, dst_writes, tag):
    """Token-major tiles [128, D] -> feature-major via [128,128] PE transposes
    packed four-at-a-time into one [128,512] psum; dst_writes(k, i0, ps) evicts."""
    for i0 in range(0, BL, 4):
        for k in range(KD):
            ps = tpsum.tile([128, 512], BF16, name=f"{tag}_tp4", tag="tp4")
            for i in range(i0, i0 + 4):
                nc.tensor.transpose(ps[:, (i - i0) * 128 : (i - i0 + 1) * 128],
                                    h_tm[i][:, k * 128 : (k + 1) * 128], ident)
            dst_writes(k, i0, ps)


def _mm_accum(nc, ps, pairs, perf_mode=None):
    n = len(pairs)
    for i, (lhsT, rhs) in enumerate(pairs):
        nc.tensor.matmul(ps, lhsT, rhs, start=(i == 0), stop=(i == n - 1),
                         perf_mode=perf_mode)


def build_program(nonzero_bo, nonzero_b2, inv_s, n_layers=L, split_waits=True):
    """inv_s: dict of per-layer descale lists: q, k, v, o (floats)."""
    nc = bass.Bass()
    x_in = nc.declare_dram_parameter("x", [T, D], F32, isOutput=False)
    y_out = nc.declare_dram_parameter("y", [T, D], F32, isOutput=True)
    kv0 = nc.declare_dram_parameter("kv0", [KP, 128, 2, TC0], F8, isOutput=False)
    wq_d = nc.declare_dram_parameter("wq", [L, MD, 128, KP, 2, 128], F8, isOutput=False)
    wk_d = nc.declare_dram_parameter("wk", [L, MD, 128, KP, 2, 128], F8, isOutput=False)
    wv_d = nc.declare_dram_parameter("wv", [L, KP, 128, 2, D], F8, isOutput=False)
    wo_d = nc.declare_dram_parameter("wo", [L, KP, 128, 2, D], F8, isOutput=False)
    w1_d = nc.declare_dram_parameter("w1", [L, FT, 128, KD * 128], BF16, isOutput=False)
    w2_d = nc.declare_dram_parameter("w2", [L, FT, 128, D], BF16, isOutput=False)
    bq_d = nc.declare_dram_parameter("bq", [L, 128, MD], F32, isOutput=False)
    b1_d = nc.declare_dram_parameter("b1", [L, 128, FT], F32, isOutput=False)
    if nonzero_bo:
        bo_d = nc.declare_dram_parameter("bo_bc", [L, 128, D], F32, isOutput=False)
    if nonzero_b2:
        b2_d = nc.declare_dram_parameter("b2_bc", [L, 128, D], F32, isOutput=False)

    with tile.TileContext(nc) as tc, ExitStack() as top:
        const = top.enter_context(tc.tile_pool(name="const", bufs=1))
        ident_bf = const.tile([128, 128], BF16, name="ident_bf")
        make_identity(nc, ident_bf)

        xpool = top.enter_context(tc.tile_pool(name="xres", bufs=BL))
        xt = []
        for i in range(BL):
            t_ = xpool.tile([128, D], F32, name=f"x{i}", tag="x")
            nc.sync.dma_start(t_, x_in[i * 128 : (i + 1) * 128, :])
            xt.append(t_)

        for l in range(n_layers):
            isq, isk, isv, iso = (inv_s[c][l] for c in "qkvo")
            # ---------------- attention sublayer ----------------
            with ExitStack() as actx:
                stats = actx.enter_context(tc.tile_pool(name=f"l{l}_st", bufs=1))
                h1tm_p = actx.enter_context(tc.tile_pool(name=f"l{l}_h1tm", bufs=4))
                h1fm_p = actx.enter_context(tc.tile_pool(name=f"l{l}_h1fm", bufs=1))
                wqk_p = actx.enter_context(tc.tile_pool(name=f"l{l}_wqk", bufs=16))
                wvo_p = actx.enter_context(tc.tile_pool(name=f"l{l}_wvo", bufs=8))
                gbufs = 1 if l == 0 else 2
                qa_p = actx.enter_context(tc.tile_pool(name=f"l{l}_qa", bufs=gbufs))
                kg_p = actx.enter_context(tc.tile_pool(name=f"l{l}_kg", bufs=gbufs))
                v_p = actx.enter_context(tc.tile_pool(name=f"l{l}_v", bufs=8))
                ag_p = actx.enter_context(tc.tile_pool(name=f"l{l}_ag", bufs=gbufs))
                sm_p = actx.enter_context(tc.tile_pool(name=f"l{l}_sm", bufs=4))
                sc_p = actx.enter_context(tc.tile_pool(name=f"l{l}_sc", bufs=4))
                ot_p = actx.enter_context(tc.tile_pool(name=f"l{l}_ot", bufs=3))
                bias_p = actx.enter_context(tc.tile_pool(name=f"l{l}_bias", bufs=1))
                ppsum = actx.enter_context(
                    tc.tile_pool(name=f"l{l}_pps", bufs=2, space="PSUM"))
                spsum = actx.enter_context(
                    tc.tile_pool(name=f"l{l}_sps", bufs=2 if l == 0 else 3,
                                 space="PSUM"))
                tpsum = actx.enter_context(
                    tc.tile_pool(name=f"l{l}_tps", bufs=2, space="PSUM"))
                apsum = actx.enter_context(
                    tc.tile_pool(name=f"l{l}_aps", bufs=2 if l == 0 else 1,
                                 space="PSUM"))
                if l == 0:
                    kv0_p = actx.enter_context(tc.tile_pool(name="l0_kv0", bufs=KP))

                bqt = bias_p.tile([128, MD], F32, name=f"l{l}_bqt")
                nc.sync.dma_start(bqt, bq_d[l])

                # LayerNorm 1 (token-major bf16) -> transpose -> fp8 pair tile
                h1tm = _layer_norm(nc, stats, xt, h1tm_p, BF16, f"l{l}a")
                hf1 = h1fm_p.tile([128, KP, 2, T], F8, name=f"l{l}_hf1", tag="hf1")

                def _hf1_write(k, i0, ps):
                    nc.vector.tensor_copy(
                        hf1[:, k // 2, k % 2, i0 * 128 : (i0 + 4) * 128], ps)

                _transpose_quads(nc, tpsum, h1tm, ident_bf, _hf1_write, f"l{l}a")

                # weight tiles for the whole layer (fp8, DoubleRow pair layout)
                wq_t = []
                wk_t = []
                for m in range(MD):
                    w = wqk_p.tile([128, KP, 2, 128], F8, name=f"l{l}_wq{m}", tag="wqk")
                    nc.sync.dma_start(w, wq_d[l, m])
                    wq_t.append(w)
                for m in range(MD):
                    w = wqk_p.tile([128, KP, 2, 128], F8, name=f"l{l}_wk{m}", tag="wqk")
                    nc.sync.dma_start(w, wk_d[l, m])
                    wk_t.append(w)
                wv_t = []
                wo_t = []
                for k in range(KP):
                    w = wvo_p.tile([128, 2, D], F8, name=f"l{l}_wv{k}", tag="wvo")
                    nc.sync.dma_start(w, wv_d[l, k])
                    wv_t.append(w)
                for k in range(KP):
                    w = wvo_p.tile([128, 2, D], F8, name=f"l{l}_wo{k}", tag="wvo")
                    nc.sync.dma_start(w, wo_d[l, k])
                    wo_t.append(w)

                TCB = C if l == 0 else S          # context length per example
                TCG = GE * TCB                    # per group

                # ---- per-group state ----
                qg_t = [None] * NGRP
                kg_t = [None] * NGRP
                ag_t = [None] * NGRP
                vts_t = [None] * NGRP

                def emit_qkv(g):
                    """Generator of per-psum emit closures for Q/K/V of group g."""
                    gcol = slice(g * GT, (g + 1) * GT)
                    if l == 0:
                        kvg = []
                        for k in range(KP):
                            kt = kv0_p.tile([128, 2, GC], F8, name=f"kv0_{k}",
                                            tag="kv0")
                            nc.sync.dma_start(kt, kv0[k, :, :, g * GC : (g + 1) * GC])
                            kvg.append(kt)
                    else:
                        kvg = None

                    qg = qa_p.tile([128, MD, GT], BF16, name=f"l{l}g{g}_q", tag="qg")
                    kg = kg_p.tile([128, MD, TCG], BF16, name=f"l{l}g{g}_k", tag="kg")
                    qg_t[g] = qg
                    kg_t[g] = kg
                    closures = []
                    for m in range(MD):
                        def _q(m=m):
                            ps = ppsum.tile([128, 512], F32, name="qps", tag="pps")
                            _mm_accum(nc, ps,
                                      [(wq_t[m][:, k], hf1[:, k, :, gcol])
                                       for k in range(KP)], DR)
                            nc.scalar.activation(qg[:, m, :], ps, ACT.Identity,
                                                 bias=bqt[:, m : m + 1], scale=isq)
                        closures.append(_q)
                    for m in range(MD):
                        for n0 in range(0, TCG, 512):
                            def _k(m=m, n0=n0):
                                n1 = min(n0 + 512, TCG)
                                ps = ppsum.tile([128, 512], F32, name="kps", tag="pps")
                                if l == 0:
                                    rh = [(wk_t[m][:, k], kvg[k][:, :, n0:n1])
                                          for k in range(KP)]
                                else:
                                    rh = [(wk_t[m][:, k],
                                           hf1[:, k, :, g * GT + n0 : g * GT + n1])
                                          for k in range(KP)]
                                _mm_accum(nc, ps[:, : n1 - n0], rh, DR)
                                nc.vector.tensor_scalar(kg[:, m, n0:n1],
                                                        ps[:, : n1 - n0],
                                                        isk, None, OP.mult)
                            closures.append(_k)
                    vts = []
                    for e in range(GE):
                        segs = []
                        for s0 in range(0, TCB, 128):
                            nrows = min(128, TCB - s0)
                            vt = v_p.tile([128, D], BF16, name=f"l{l}g{g}e{e}v{s0}",
                                          tag="v")
                            segs.append((vt, nrows))
                        vts.append(segs)
                    vts_t[g] = vts
                    for e in range(GE):
                        for si, (vt, nrows) in enumerate(vts[e]):
                            s0 = si * 128
                            for n in range(2):
                                def _v(e=e, s0=s0, nrows=nrows, vt=vt, n=n):
                                    ps = ppsum.tile([128, 512], F32, name="vps",
                                                    tag="pps")
                                    if l == 0:
                                        lh = [(kvg[k][:, :, e * TCB + s0 :
                                                       e * TCB + s0 + nrows],
                                               wv_t[k][:, :, n * 512 : (n + 1) * 512])
                                              for k in range(KP)]
                                    else:
                                        c0 = (g * GE + e) * 128
                                        lh = [(hf1[:, k, :, c0 : c0 + 128],
                                               wv_t[k][:, :, n * 512 : (n + 1) * 512])
                                              for k in range(KP)]
                                    _mm_accum(nc, ps[: nrows], lh, DR)
                                    nc.vector.tensor_scalar(
                                        vt[:nrows, n * 512 : (n + 1) * 512],
                                        ps[:nrows], isv, None, OP.mult)
                                closures.append(_v)
                    return closures

                def emit_o(g):
                    """Per-psum closures for output projection + residual."""
                    ag = ag_t[g]
                    closures = []
                    for e in range(GE):
                        xi = xt[g * GE + e]
                        for n in range(2):
                            def _o(e=e, xi=xi, n=n):
                                ps = ppsum.tile([128, 512], F32, name="ops", tag="pps")
                                _mm_accum(nc, ps,
                                          [(ag[:, k, :, e * 128 : (e + 1) * 128],
                                            wo_t[k][:, :, n * 512 : (n + 1) * 512])
                                           for k in range(KP)], DR)
                                xs = xi[:, n * 512 : (n + 1) * 512]
                                ot = ot_p.tile([128, 512], F32, name="ot", tag="ot")
                                nc.scalar.activation(ot, ps, ACT.Copy, scale=iso)
                                nc.gpsimd.tensor_tensor(xs, xs, ot, OP.add)
                            closures.append(_o)
                    return closures

                # ---- attention smalls: software-pipelined blocks ----
                # Per block (example e, bh consecutive heads):
                #   front: per-head scores matmul (K=64, offset-0 psum) ->
                #          exp on ACT with accum_out row-sums -> recip (DVE)
                #          -> normalize (Pool, free-dim-broadcast rinv)
                #   back:  p-transposes (K=128, packed psum) -> pts copy (DVE)
                #          -> AV matmuls -> ag eviction (ACT, fp8 cast)
                # Blocks run SKEW apart so PE never waits on the softmax chain;
                # projection psums of the other group interleave between blocks.
                bh = 512 // TCB          # 4 heads (S=128) or 2 heads (C=196)
                nseg = (TCB + 127) // 128
                SKEW = 2

                def smalls_front(g, e, hb):
                    qg, kg = qg_t[g], kg_t[g]
                    praw = sm_p.tile([128, bh, TCB], BF16, name="praw", tag="praw")
                    ssum = sc_p.tile([128, bh], F32, name="ssum", tag="ssum")
                    for hi in range(bh):
                        h_ = hb + hi
                        po, ch = 64 * (h_ % 2), h_ // 2
                        sp = spsum.tile([128, TCB], F32, name="sp", tag="sps")
                        nc.tensor.matmul(
                            sp,
                            qg[po : po + 64, ch, e * 128 : (e + 1) * 128],
                            kg[po : po + 64, ch, e * TCB : (e + 1) * TCB],
                            start=True, stop=True)
                        # exp without max-subtraction (|scores| << 80); row sums
                        # accumulate on the fly
                        if USE_ACCUM_EXP:
                            nc.scalar.activation(praw[:, hi, :], sp, ACT.Exp,
                                                 accum_out=ssum[:, hi : hi + 1])
                        else:
                            nc.scalar.activation(praw[:, hi, :], sp, ACT.Exp)
                    if not USE_ACCUM_EXP:
                        nc.vector.tensor_reduce(ssum, praw, AX, OP.add)
                    rinv = sc_p.tile([128, bh], F32, name="rinv", tag="rinv")
                    nc.vector.reciprocal(rinv, ssum)
                    pbf = sm_p.tile([128, bh, TCB], BF16, name="pbf", tag="pbf")
                    eng = nc.gpsimd if USE_POOL_NORM else nc.vector
                    eng.tensor_tensor(
                        pbf, praw,
                        rinv[:, :, None].broadcast_to((128, bh, TCB)),
                        OP.mult)
                    return pbf

                def smalls_back1(g, e, hb, pbf):
                    tp4 = tpsum.tile([128, nseg, bh, 128], BF16,
                                     name="ptp4", tag="tp4")
                    for hi in range(bh):
                        for si in range(nseg):
                            nrows = min(128, TCB - si * 128)
                            nc.tensor.transpose(
                                tp4[:nrows, si, hi, :],
                                pbf[:, hi, si * 128 : si * 128 + nrows],
                                ident_bf)
                    pts = sm_p.tile([128, nseg, bh, 128], BF16,
                                    name="pts", tag="pts")
                    for si in range(nseg):
                        nrows = min(128, TCB - si * 128)
                        nc.vector.tensor_copy(pts[:nrows, si], tp4[:nrows, si])
                    return pts

                def smalls_back2(g, e, hb, pts):
                    ag, vts = ag_t[g], vts_t[g]
                    if l > 0 and bh == 4 and USE_PACKED_APS:
                        # packed AV psum: 4 heads in one bank, all at partition
                        # offset 0 (K=128 col-offset writes are safe; partition
                        # -offset matmul writes hard-fault this silicon).
                        # Slot order groups the po=0 heads (hb, hb+2) in slots
                        # 0:2 and po=64 heads in slots 2:4 so each ag half
                        # evicts as one contiguous [64, 2, 128] activation.
                        aps = apsum.tile([64, 4, 128], F32, name="aps", tag="aps")
                        for hi in range(bh):
                            h_ = hb + hi
                            slot = (hi % 2) * 2 + hi // 2
                            vt, nrows = vts[e][0]
                            nc.tensor.matmul(
                                aps[:, slot, :],
                                vt[:nrows, h_ * 64 : (h_ + 1) * 64],
                                pts[:nrows, 0, hi, :],
                                start=True, stop=True)
                        ecol = slice(e * 128, (e + 1) * 128)
                        nc.scalar.activation(
                            ag[0:64, hb // 4, :, ecol], aps[:, 0:2, :], ACT.Copy)
                        nc.scalar.activation(
                            ag[64:128, hb // 4, :, ecol], aps[:, 2:4, :], ACT.Copy)
                    else:
                        for hi in range(bh):
                            h_ = hb + hi
                            po, ch = 64 * (h_ % 2), h_ // 2
                            aps = apsum.tile([64, 128], F32, name="aps0", tag="aps")
                            for si in range(nseg):
                                nrows = min(128, TCB - si * 128)
                                vt, _ = vts[e][si]
                                nc.tensor.matmul(
                                    aps, vt[:nrows, h_ * 64 : (h_ + 1) * 64],
                                    pts[:nrows, si, hi, :],
                                    start=(si == 0), stop=(si == nseg - 1))
                            nc.scalar.activation(
                                ag[po : po + 64, ch // 2, ch % 2,
                                   e * 128 : (e + 1) * 128],
                                aps, ACT.Copy)

                def smalls_group(g, fillers):
                    """Pipelined smalls for group g, interleaving `fillers`
                    (projection psum closures) between blocks.  Transposes run
                    SKEW slots behind scores; AV one more slot behind so the
                    pts SBUF copy has a full slot to land."""
                    ag_t[g] = ag_p.tile([128, KP, 2, GT], F8,
                                        name=f"l{l}g{g}_a", tag="ag")
                    blocks = [(e, hb) for e in range(GE) for hb in range(0, H, bh)]
                    fq = list(fillers)
                    fi = 0
                    nfill = len(fq)
                    nb = len(blocks)
                    nslots = nb + SKEW + 1
                    pend1 = []
                    pend2 = []
                    for bi in range(nslots):
                        if bi < nb:
                            e, hb = blocks[bi]
                            pend1.append((e, hb, smalls_front(g, e, hb)))
                        want = nfill * (bi + 1) // nslots
                        while fi < want:
                            fq[fi]()
                            fi += 1
                        if bi >= SKEW and pend1:
                            e, hb, pbf = pend1.pop(0)
                            pend2.append((e, hb, smalls_back1(g, e, hb, pbf)))
                        if bi >= SKEW + 1 and pend2:
                            e, hb, pts = pend2.pop(0)
                            smalls_back2(g, e, hb, pts)

                # ---- layer schedule ----
                qkv0 = emit_qkv(0)
                for c in qkv0:
                    c()
                qkv1 = emit_qkv(1)
                smalls_group(0, qkv1)
                smalls_group(1, emit_o(0))
                for c in emit_o(1):
                    c()
                if nonzero_bo:
                    bo_t = bias_p.tile([128, D], F32, name=f"l{l}_bo")
                    nc.sync.dma_start(bo_t, bo_d[l])
                    for i in range(BL):
                        nc.vector.tensor_tensor(xt[i], xt[i], bo_t, OP.add)

            # ---------------- FFN sublayer ----------------
            with ExitStack() as fctx:
                stats2 = fctx.enter_context(tc.tile_pool(name=f"l{l}_st2", bufs=1))
                h2tm_p = fctx.enter_context(tc.tile_pool(name=f"l{l}_h2tm", bufs=4))
                h2fm_p = fctx.enter_context(tc.tile_pool(name=f"l{l}_h2fm", bufs=KD))
                w1_p = fctx.enter_context(tc.tile_pool(name=f"l{l}_w1", bufs=12))
                w2_p = fctx.enter_context(tc.tile_pool(name=f"l{l}_w2", bufs=12))
                u_p = fctx.enter_context(tc.tile_pool(name=f"l{l}_u", bufs=12))
                bias2_p = fctx.enter_context(tc.tile_pool(name=f"l{l}_b2", bufs=1))
                fpsum = fctx.enter_context(
                    tc.tile_pool(name=f"l{l}_fps", bufs=4, space="PSUM"))
                tpsum2 = fctx.enter_context(
                    tc.tile_pool(name=f"l{l}_tps2", bufs=2, space="PSUM"))

                b1t = bias2_p.tile([128, FT], F32, name=f"l{l}_b1t")
                nc.sync.dma_start(b1t, b1_d[l])

                h2tm = _layer_norm(nc, stats2, xt, h2tm_p, BF16, f"l{l}f")
                h2fm = [h2fm_p.tile([128, T], BF16, name=f"l{l}f_fm{k}", tag="h2fm")
                        for k in range(KD)]

                def _h2_write(k, i0, ps):
                    nc.vector.tensor_copy(h2fm[k][:, i0 * 128 : (i0 + 4) * 128], ps)

                _transpose_quads(nc, tpsum2, h2tm, ident_bf, _h2_write, f"l{l}f")

                for fb in range(NFB):
                    w1t = []
                    w2t = []
                    for ft_ in range(FBT):
                        w = w1_p.tile([128, KD * 128], BF16,
                                      name=f"l{l}fb{fb}w1_{ft_}", tag="w1")
                        nc.sync.dma_start(w, w1_d[l, fb * FBT + ft_])
                        w1t.append(w)
                        w_ = w2_p.tile([128, D], BF16,
                                       name=f"l{l}fb{fb}w2_{ft_}", tag="w2")
                        nc.sync.dma_start(w_, w2_d[l, fb * FBT + ft_])
                        w2t.append(w_)
                    for th in range(2):
                        tcol = slice(th * 512, (th + 1) * 512)
                        uts = []
                        for ft_ in range(FBT):
                            ps = fpsum.tile([128, 512], F32, name="ups", tag="fps")
                            _mm_accum(nc, ps,
                                      [(w1t[ft_][:, k * 128 : (k + 1) * 128],
                                        h2fm[k][:, tcol]) for k in range(KD)])
                            ut = u_p.tile([128, 512], BF16,
                                          name=f"u{fb}_{th}_{ft_}", tag="u")
                            nc.scalar.activation(
                                ut, ps, ACT.Gelu_apprx_tanh,
                                bias=b1t[:, fb * FBT + ft_ : fb * FBT + ft_ + 1])
                            uts.append(ut)
                        for m in range(4):
                            xi = xt[th * 4 + m]
                            for n in range(2):
                                ps = fpsum.tile([128, 512], F32, name="yps", tag="fps")
                                _mm_accum(nc, ps,
                                          [(uts[kf][:, m * 128 : (m + 1) * 128],
                                            w2t[kf][:, n * 512 : (n + 1) * 512])
                                           for kf in range(FBT)])
                                xs = xi[:, n * 512 : (n + 1) * 512]
                                if fb == NFB - 1 and th == 1:
                                    # keep the DVE FIFO clear at the layer
                                    # boundary: next layer's LN1 stats must not
                                    # queue behind these evictions
                                    ot = u_p.tile([128, 512], F32, name="fot",
                                                  tag="fot")
                                    nc.scalar.activation(ot, ps, ACT.Copy)
                                    nc.gpsimd.tensor_tensor(xs, xs, ot, OP.add)
                                else:
                                    nc.vector.tensor_tensor(xs, xs, ps, OP.add)
                if nonzero_b2:
                    b2_t = bias2_p.tile([128, D], F32, name=f"l{l}_b2bc")
                    nc.sync.dma_start(b2_t, b2_d[l])
                    for i in range(BL):
                        nc.vector.tensor_tensor(xt[i], xt[i], b2_t, OP.add)

        for i in range(BL):
            nc.sync.dma_start(y_out[i * 128 : (i + 1) * 128, :], xt[i])

    if split_waits:
        _split_multi_waits(nc)
    return nc


def _q8(w, s):
    """host-side e4m3 (TRN variant, max 240) quantization of w*s"""
    e4m3 = ml_dtypes.float8_e4m3
    return np.clip(np.asarray(w, np.float32) * s, -240.0, 240.0).astype(e4m3)


def prepare_host(inputs, n_layers=L):
    """Fold LN affines + biases into weights; arrange DMA-friendly layouts."""
    f32 = np.float32
    bf16 = ml_dtypes.bfloat16
    e4m3 = ml_dtypes.float8_e4m3
    Wq = np.asarray(inputs["Wq"], f32)
    Wk = np.asarray(inputs["Wk"], f32)
    Wv = np.asarray(inputs["Wv"], f32)
    Wo = np.asarray(inputs["Wo"], f32)
    W1 = np.asarray(inputs["W1"], f32)
    W2 = np.asarray(inputs["W2"], f32)
    bq = np.asarray(inputs["bq"], f32)
    bk = np.asarray(inputs["bk"], f32)   # dropped: softmax row-shift invariance
    bv = np.asarray(inputs["bv"], f32)
    bo = np.asarray(inputs["bo"], f32)
    b1 = np.asarray(inputs["b1"], f32)
    b2 = np.asarray(inputs["b2"], f32)
    g1 = np.asarray(inputs["ln1_g"], f32)
    be1 = np.asarray(inputs["ln1_b"], f32)
    g2 = np.asarray(inputs["ln2_g"], f32)
    be2 = np.asarray(inputs["ln2_b"], f32)

    scale = np.float32(1.0 / np.sqrt(DK))
    Wq_e = (g1[:, :, None] * Wq) * scale
    bq_e = (bq + np.einsum("ld,ldo->lo", be1, Wq)) * scale
    Wk_e = Wk.copy()
    Wv_e = Wv.copy()
    bv_e = bv.copy()
    for l in range(1, L):
        Wk_e[l] = g1[l][:, None] * Wk[l]
        Wv_e[l] = g1[l][:, None] * Wv[l]
        bv_e[l] = bv[l] + be1[l] @ Wv[l]
    bo_e = bo + np.einsum("ld,ldo->lo", bv_e, Wo)
    W1_e = g2[:, :, None] * W1
    b1_e = b1 + np.einsum("ld,ldo->lo", be2, W1)

    # per-layer fp8 weight scales (e4m3 abs-max target 192; TRN max 240)
    def _scales(w):
        return [np.float32(192.0 / max(np.abs(w[l]).max(), 1e-8))
                for l in range(L)]

    s_q, s_k, s_v, s_o = (_scales(w) for w in (Wq_e, Wk_e, Wv_e, Wo))
    inv_s = {
        "q": [float(1.0 / s) for s in s_q],
        "k": [float(1.0 / s) for s in s_k],
        "v": [float(1.0 / s) for s in s_v],
        "o": [float(1.0 / s) for s in s_o],
    }

    def colblocks(w):  # [L, D_in, N] -> [L, N/128, 128, (D_in/128)*128]
        kd = w.shape[1] // 128
        nt = w.shape[2] // 128
        return np.ascontiguousarray(
            w.reshape(L, kd, 128, nt, 128).transpose(0, 3, 2, 1, 4)
        ).reshape(L, nt, 128, kd * 128)

    def rowpairs(w):   # [L, D_in, N] -> [L, KP, 128, 2, N]
        return np.ascontiguousarray(
            w.reshape(L, KP, 2, 128, w.shape[2]).transpose(0, 1, 3, 2, 4))

    sq_a = np.asarray(s_q, f32)[:, None, None]
    sk_a = np.asarray(s_k, f32)[:, None, None]
    sv_a = np.asarray(s_v, f32)[:, None, None]
    so_a = np.asarray(s_o, f32)[:, None, None]

    host = {
        "wq": _q8(colblocks(Wq_e * sq_a), 1.0).reshape(L, MD, 128, KP, 2, 128),
        "wk": _q8(colblocks(Wk_e * sk_a), 1.0).reshape(L, MD, 128, KP, 2, 128),
        "wv": _q8(rowpairs(Wv_e * sv_a), 1.0),
        "wo": _q8(rowpairs(Wo * so_a), 1.0),
        "w1": colblocks(W1_e).astype(bf16),
        "w2": np.ascontiguousarray(W2.reshape(L, FT, 128, D)).astype(bf16),
        "bq": np.ascontiguousarray(bq_e.reshape(L, MD, 128).transpose(0, 2, 1)),
        "b1": np.ascontiguousarray(b1_e.reshape(L, FT, 128).transpose(0, 2, 1)),
    }
    nonzero_bo = bool(np.any(bo_e))
    nonzero_b2 = bool(np.any(b2))
    if nonzero_bo:
        host["bo_bc"] = np.ascontiguousarray(
            np.broadcast_to(bo_e[:, None, :], (L, 128, D)).astype(f32))
    if nonzero_b2:
        host["b2_bc"] = np.ascontiguousarray(
            np.broadcast_to(b2[:, None, :], (L, 128, D)).astype(f32))

    xt = np.asarray(inputs["xt"], f32)
    p_att = np.asarray(inputs["p_att_feats"], f32)
    per_core = []
    for c in range(NCORES):
        xs = np.ascontiguousarray(xt[c * BL : (c + 1) * BL].reshape(T, D))
        kv = p_att[c * BL : (c + 1) * BL].transpose(2, 0, 1).reshape(KD, 128, TC0)
        kv = np.ascontiguousarray(
            kv.reshape(KP, 2, 128, TC0).transpose(0, 2, 1, 3))
        m = dict(host)
        m["x"] = xs
        m["kv0"] = np.clip(kv, -240.0, 240.0).astype(e4m3)
        per_core.append(m)
    return per_core, nonzero_bo, nonzero_b2, inv_s


def run(inputs, n_layers=L, trace=False, trace_dir=None):
    per_core, nz_bo, nz_b2, inv_s = prepare_host(inputs, n_layers)
    nc = build_program(nz_bo, nz_b2, inv_s, n_layers)
    res = run_bass_kernel_spmd(nc, per_core, list(range(NCORES)))
    out = np.empty((B, S, D), np.float32)
    for c in range(NCORES):
        out[c * BL : (c + 1) * BL] = res.results[c]["y"].reshape(BL, S, D)
    return out


def kernel(**inputs) -> np.ndarray:
    return run(inputs)
